# revision 17
# baseline (speedup 1.0000x reference)
"""Trainium2 Bass kernel for LocalEnvironmentEmbedding (GNN message passing).

Math (per edge e with src s, dst d):
    feats   = [node_attr[s], node_attr[d], edge_embed[e]]          # [192]
    es      = feats @ (W_lin / sqrt(192))                          # [64]
    h1      = silu_n(es @ W1/8); h2 = silu_n(h1 @ W2/8)
    w       = h2 @ W3/8                                            # [64]
    out[e]  = concat_b( outer(w[16b:16b+16], attr_block_b) )       # [256]

W_lin and W1 compose linearly (no activation between them), so the host
folds them into Wc = W_lin' @ W1' [192, 64].  The per-node halves of that
product, u[n] = node[n] @ Wc[:64] and v[n] = node[n] @ Wc[64:128], are
precomputed per node (40000x64x64, ~1% of total FLOPs) and the host
streams uv[e] = u[src[e]] + v[dst[e]] per edge, avoiding the slow
device-side row gather.  All per-edge compute (emb projection, both
hidden layers, final linear, tensor-product expansion) runs on device.

Distribution: edges sharded across 8 cores (80000 each), no cross-device
communication.  Streams are fp16 (PSUM accumulation stays f32); the
device writes fp16 output which the host upcasts to f32.

Device layout per 1024-edge tile (edge slot n = 8p + c for partition p,
chunk c in [0,8); half H = p//64 -- half H's hidden vectors live on
partitions [64H, 64H+64), so every matmul is a (0,0)/(0,64)/(64,64)
PE tile; other tile-position mixes wedge the device):
  - in stream [128, 1152]: cols 512H..512H+512 hold half H's moving
    operand (rows 0:64 uv feats, rows 64:128 emb feats), cols 1024:1152
    hold attr edge-on-partition [128, 8, 16]
  - h1[64H:64H+64] = W_ue^T @ in[:, 512H:...]  (one K=128 matmul/half)
  - silu on [128, 512] (all lanes), h2 likewise with half-replicated W2'
  - final layer: h2 [64, 64] chunks stationary x W3' moving -> w back
    in edge-on-partition PSUM [128, 8, 64] (diagonal tiles only)
  - output expansion: DVE broadcast multiplies into [128, 8, 256] fp16
  - out rows e = 8p + c give each partition a 4 KB contiguous HBM span
"""

import numpy as np

import concourse.bass as bass
import concourse.tile as tile
from concourse import bacc, mybir
from concourse.bass_utils import run_bass_kernel_spmd

F32 = mybir.dt.float32
F16 = mybir.dt.float16
AF = mybir.ActivationFunctionType

_SILU_NORM = 1.679177

N_CORES = 8
E_TOTAL = 640000
E_CORE = E_TOTAL // N_CORES
P = 128
T = 1024                       # edges per tile
NT = (E_CORE + T - 1) // T     # 79 tiles
E_PAD = NT * T

# (16-col weight block, attr dim d, attr col offset, out col offset)
BLOCKS = [(0, 1, 0, 0), (1, 3, 1, 16), (2, 5, 4, 64), (3, 7, 9, 144)]


def build_nc(nt: int = NT):
    nc = bacc.Bacc()

    in_p = nc.declare_dram_parameter("in", [nt, P, 1152], F16, isOutput=False)
    wts_p = nc.declare_dram_parameter("wts", [P, 3, 64], F16, isOutput=False)
    # output split so DVE (blocks d3,d7 -> [48|112] cols) and GpSimd (blocks
    # d1,d5 -> [16|80] cols) write independent tiles and run in parallel --
    # a shared tile serializes the writers across engines; host reassembles
    outa_p = nc.declare_dram_parameter("outa", [nt, T, 160], F16, isOutput=True)
    outb_p = nc.declare_dram_parameter("outb", [nt, T, 96], F16, isOutput=True)

    with tile.TileContext(nc) as tc:
        with (
            tc.tile_pool(name="singles", bufs=1) as singles,
            tc.tile_pool(name="ins", bufs=4) as ipool,
            tc.tile_pool(name="acts", bufs=3) as hpool,
            tc.tile_pool(name="outs", bufs=4) as opool,
            tc.tile_pool(name="ps_h", bufs=2, space="PSUM") as mpool,
            tc.tile_pool(name="ps_w", bufs=3, space="PSUM") as wpool,
        ):
            wts_sb = singles.tile([P, 3, 64], F16)
            nc.sync.dma_start(out=wts_sb[:], in_=wts_p[:])

            # issue input loads a few tiles ahead so they sit in front of
            # earlier tiles' output stores in the sync queue (the store at
            # the queue head blocks on that tile's compute, which would
            # otherwise stall all later loads)
            LOOKAHEAD = 3
            in_tiles = {}

            def load(t):
                if t < nt:
                    in_tiles[t] = ipool.tile([P, 1152], F16, tag="in",
                                             name=f"in_sb_{t}")
                    nc.sync.dma_start(out=in_tiles[t][:], in_=in_p[t])

            for t in range(LOOKAHEAD):
                load(t)

            for t in range(nt):
                in_sb = in_tiles.pop(t)

                h1_ps = mpool.tile([P, 512], F32, tag="h1")
                for h in range(2):
                    nc.tensor.matmul(h1_ps[64 * h:64 * h + 64, :], wts_sb[:, 0, :],
                                     in_sb[:, 512 * h:512 * h + 512],
                                     start=True, stop=True)
                h1_sb = hpool.tile([P, 512], F16, tag="h1s")
                nc.scalar.activation(h1_sb[:], h1_ps[:], AF.Silu)

                h2_ps = mpool.tile([P, 512], F32, tag="h2")
                for h in range(2):
                    hs = slice(64 * h, 64 * h + 64)
                    nc.tensor.matmul(h2_ps[hs, :], wts_sb[hs, 1, :], h1_sb[hs, :],
                                     start=True, stop=True)
                h2_sb = hpool.tile([P, 512], F16, tag="h2s")
                nc.scalar.activation(h2_sb[:], h2_ps[:], AF.Silu)

                # final layer: diagonal PE tiles only ((0,0) and (64,64)) --
                # mixing other tile positions back-to-back wedges the device
                w_ps = wpool.tile([P, 8, 64], F32, tag="w")
                for h in range(2):
                    hs = slice(64 * h, 64 * h + 64)
                    for c in range(8):
                        nc.tensor.matmul(w_ps[hs, c, :],
                                         h2_sb[hs, 64 * c:64 * c + 64],
                                         wts_sb[hs, 2, :], start=True, stop=True)

                # GpSimd cannot read PSUM, so Scalar lands its w cols (blocks
                # d1 and d5: 0:16 and 32:48) in SBUF via one strided copy
                wg_sb = hpool.tile([P, 8, 2, 16], F16, tag="wg")
                w_head = w_ps[:, :, 0:16]
                w_src = bass.AP(tensor=w_head.tensor, offset=w_head.offset,
                                ap=list(w_head.ap[:2]) + [[32, 2], [1, 16]])
                nc.scalar.copy(wg_sb[:], w_src)

                outa_sb = opool.tile([P, 8, 160], F16, tag="outa")
                outb_sb = opool.tile([P, 8, 96], F16, tag="outb")
                attr_ap = in_sb[:, 1024:1152].rearrange("p (c k) -> p c k", k=16)

                def expand(eng, o_sb, o_off, w_sl, d, aoff):
                    o_ap = o_sb[:, :, o_off:o_off + 16 * d].rearrange(
                        "p c (j k) -> p c j k", k=d)
                    w_ap = bass.AP(tensor=w_sl.tensor, offset=w_sl.offset,
                                   ap=list(w_sl.ap) + [[0, d]])
                    a_sl = attr_ap[:, :, aoff:aoff + d]
                    a_ap = bass.AP(tensor=a_sl.tensor, offset=a_sl.offset,
                                   ap=list(a_sl.ap[:2]) + [[0, 16]] + list(a_sl.ap[2:]))
                    eng.tensor_mul(o_ap, w_ap, a_ap)

                expand(nc.gpsimd, outb_sb, 16, wg_sb[:, :, 1, :], 5, 4)
                expand(nc.gpsimd, outb_sb, 0, wg_sb[:, :, 0, :], 1, 0)
                expand(nc.vector, outa_sb, 0, w_ps[:, :, 16:32], 3, 1)
                expand(nc.vector, outa_sb, 48, w_ps[:, :, 48:64], 7, 9)

                load(t + LOOKAHEAD)
                nc.sync.dma_start(out=outa_p[t].rearrange("(p c) f -> p c f", p=P),
                                  in_=outa_sb[:])
                nc.sync.dma_start(out=outb_p[t].rearrange("(p c) f -> p c f", p=P),
                                  in_=outb_sb[:])

    nc.compile()
    return nc


def prep_weights(W_lin, W1, W2, W3):
    """Host weight prep: fold W_lin@W1, silu-norm into W2/W3, fp16 pack."""
    Wc = (W_lin.astype(np.float64) / np.sqrt(192.0)) @ (W1.astype(np.float64) / 8.0)
    s = np.float64(_SILU_NORM / 8.0)
    W_ue = np.concatenate([np.eye(64), Wc[128:192]], axis=0)      # [128, 64]
    wts = np.empty((P, 3, 64), np.float16)
    wts[:, 0, :] = W_ue
    wts[0:64, 1, :] = W2 * s
    wts[64:128, 1, :] = W2 * s
    wts[0:64, 2, :] = W3 * s
    wts[64:128, 2, :] = W3 * s
    return wts, Wc.astype(np.float32)


_CMAP = None


def _cmap():
    """Within-tile column->edge map: half H, col j -> n = 8*(64H + j%64) + j//64.

    Half H's hidden vectors live on partitions [64H, 64H+64); its edges own
    out slots (p, c) with p in that range, so every final-layer matmul is a
    diagonal PE tile.
    """
    global _CMAP
    if _CMAP is None:
        j = np.arange(512)
        _CMAP = np.stack([8 * (64 * H + j % 64) + j // 64 for H in (0, 1)])
    return _CMAP


def prep_core_input(uv16, emb16, attr16, nt: int = NT):
    """Build one core's [nt, 128, 1152] fp16 device stream.

    uv16/emb16: [E_PAD, 64] fp16; attr16: [E_PAD, 16] fp16 (zero-padded).
    """
    cmap = _cmap()
    uv_r = uv16.reshape(nt, T, 64)[:, cmap, :].transpose(0, 3, 1, 2).reshape(nt, 64, 1024)
    emb_r = emb16.reshape(nt, T, 64)[:, cmap, :].transpose(0, 3, 1, 2).reshape(nt, 64, 1024)
    attr_r = attr16.reshape(nt, P, 8 * 16)
    return np.ascontiguousarray(
        np.concatenate([np.concatenate([uv_r, emb_r], axis=1), attr_r], axis=2))


def prep_in_maps(edge_index, node_attr, edge_attr, edge_embed, W_lin, W1, W2, W3):
    wts, Wc = prep_weights(np.asarray(W_lin, np.float32), np.asarray(W1, np.float32),
                           np.asarray(W2, np.float32), np.asarray(W3, np.float32))
    node_attr = np.asarray(node_attr, np.float32)
    idx = np.asarray(edge_index).astype(np.int64)
    u = node_attr @ Wc[0:64]
    v = node_attr @ Wc[64:128]
    uv16 = (u[idx[0]] + v[idx[1]]).astype(np.float16)              # [E, 64]
    emb16 = np.asarray(edge_embed).astype(np.float16)
    attr16 = np.asarray(edge_attr).astype(np.float16)

    in_maps = []
    for i in range(N_CORES):
        sl = slice(i * E_CORE, (i + 1) * E_CORE)
        uv_c = np.zeros((E_PAD, 64), np.float16)
        emb_c = np.zeros((E_PAD, 64), np.float16)
        attr_c = np.zeros((E_PAD, 16), np.float16)
        uv_c[:E_CORE] = uv16[sl]
        emb_c[:E_CORE] = emb16[sl]
        attr_c[:E_CORE] = attr16[sl]
        in_maps.append({"in": prep_core_input(uv_c, emb_c, attr_c), "wts": wts})
    return in_maps


def kernel(edge_index, node_attr, edge_attr, edge_embed, W_lin, W1, W2, W3):
    in_maps = prep_in_maps(edge_index, node_attr, edge_attr, edge_embed,
                           W_lin, W1, W2, W3)
    nc = build_nc()
    res = run_bass_kernel_spmd(nc, in_maps, list(range(N_CORES)))
    out = np.empty((E_TOTAL, 256), np.float32)
    for i in range(N_CORES):
        sl = slice(i * E_CORE, (i + 1) * E_CORE)
        a = res.results[i]["outa"].reshape(E_PAD, 160)[:E_CORE]
        b = res.results[i]["outb"].reshape(E_PAD, 96)[:E_CORE]
        out[sl, 0:16] = b[:, 0:16]      # d1
        out[sl, 16:64] = a[:, 0:48]     # d3
        out[sl, 64:144] = b[:, 16:96]   # d5
        out[sl, 144:256] = a[:, 48:160]  # d7
    return out


if __name__ == "__main__":
    pass


# revision 18
# speedup vs baseline: 1.0150x; 1.0150x over previous
"""Trainium2 Bass kernel for LocalEnvironmentEmbedding (GNN message passing).

Math (per edge e with src s, dst d):
    feats   = [node_attr[s], node_attr[d], edge_embed[e]]          # [192]
    es      = feats @ (W_lin / sqrt(192))                          # [64]
    h1      = silu_n(es @ W1/8); h2 = silu_n(h1 @ W2/8)
    w       = h2 @ W3/8                                            # [64]
    out[e]  = concat_b( outer(w[16b:16b+16], attr_block_b) )       # [256]

W_lin and W1 compose linearly (no activation between them), so the host
folds them into Wc = W_lin' @ W1' [192, 64].  The per-node halves of that
product, u[n] = node[n] @ Wc[:64] and v[n] = node[n] @ Wc[64:128], are
precomputed per node (40000x64x64, ~1% of total FLOPs) and the host
streams uv[e] = u[src[e]] + v[dst[e]] per edge, avoiding the slow
device-side row gather.  All per-edge compute (emb projection, both
hidden layers, final linear, tensor-product expansion) runs on device.

Distribution: edges sharded across 8 cores (80000 each), no cross-device
communication.  Streams are fp16 (PSUM accumulation stays f32); the
device writes fp16 output which the host upcasts to f32.

Device layout per 1024-edge tile (edge slot n = 8p + c for partition p,
chunk c in [0,8); half H = p//64 -- half H's hidden vectors live on
partitions [64H, 64H+64), so every matmul is a (0,0)/(0,64)/(64,64)
PE tile; other tile-position mixes wedge the device):
  - in stream (two tiles per DMA) [128, 2, 1152]: cols 512H..512H+512
    hold half H's moving operand (rows 0:64 uv feats, rows 64:128 emb
    feats), cols 1024:1152 hold attr edge-on-partition [128, 8, 16]
  - h1[64H:64H+64] = W_ue^T @ in[:, 512H:...]  (one K=128 matmul/half)
  - silu on [128, 512] (all lanes), h2 likewise with half-replicated W2'
  - final layer: h2 [64, 64] chunks stationary x W3' moving -> w back
    in edge-on-partition PSUM [128, 8, 64] (diagonal tiles only)
  - output expansion: DVE writes blocks d1,d3,d7 straight from PSUM;
    GpSimd (via a Scalar PSUM->SBUF copy of w cols 32:48) writes d5 to
    a separate tile so the two engines run in parallel; the host
    reassembles the column order
  - out rows e = 8p + c give each partition a contiguous HBM span
"""

import numpy as np

import concourse.bass as bass
import concourse.tile as tile
from concourse import bacc, mybir
from concourse.bass_utils import run_bass_kernel_spmd

F32 = mybir.dt.float32
F16 = mybir.dt.float16
AF = mybir.ActivationFunctionType

_SILU_NORM = 1.679177

N_CORES = 8
E_TOTAL = 640000
E_CORE = E_TOTAL // N_CORES
P = 128
T = 1024                       # edges per tile
NT = 80                        # tiles per core (even, for paired loads)
NPAIR = NT // 2
E_PAD = NT * T


def build_nc(nt: int = NT):
    nc = bacc.Bacc()

    in_p = nc.declare_dram_parameter("in", [nt // 2, P, 2, 1152], F16,
                                     isOutput=False)
    wts_p = nc.declare_dram_parameter("wts", [P, 3, 64], F16, isOutput=False)
    outa_p = nc.declare_dram_parameter("outa", [nt, T, 176], F16, isOutput=True)
    outb_p = nc.declare_dram_parameter("outb", [nt, T, 80], F16, isOutput=True)

    with tile.TileContext(nc) as tc:
        with (
            tc.tile_pool(name="singles", bufs=1) as singles,
            tc.tile_pool(name="ins", bufs=3) as ipool,
            tc.tile_pool(name="acts", bufs=3) as hpool,
            tc.tile_pool(name="outs", bufs=4) as opool,
            tc.tile_pool(name="ps_h", bufs=2, space="PSUM") as mpool,
            tc.tile_pool(name="ps_w", bufs=4, space="PSUM") as wpool,
        ):
            wts_sb = singles.tile([P, 3, 64], F16)
            nc.sync.dma_start(out=wts_sb[:], in_=wts_p[:])

            # issue paired input loads ahead so they sit in front of earlier
            # tiles' output stores in the sync queue (a store at the queue
            # head blocks on its tile's compute, which would otherwise stall
            # all later loads)
            LOOKAHEAD = 2
            in_pairs = {}

            def load(u):
                if u < nt // 2:
                    in_pairs[u] = ipool.tile([P, 2, 1152], F16, tag="in",
                                             name=f"in_sb_{u}")
                    nc.sync.dma_start(out=in_pairs[u][:], in_=in_p[u])

            for u in range(LOOKAHEAD):
                load(u)

            for t in range(nt):
                in_sb = in_pairs[t // 2][:, t % 2, :]
                if t % 2 == 1:
                    in_pairs.pop(t // 2)

                h1_ps = mpool.tile([P, 512], F32, tag="h1")
                for h in range(2):
                    nc.tensor.matmul(h1_ps[64 * h:64 * h + 64, :], wts_sb[:, 0, :],
                                     in_sb[:, 512 * h:512 * h + 512],
                                     start=True, stop=True)
                h1_sb = hpool.tile([P, 512], F16, tag="h1s")
                nc.scalar.activation(h1_sb[:], h1_ps[:], AF.Silu)

                h2_ps = mpool.tile([P, 512], F32, tag="h2")
                for h in range(2):
                    hs = slice(64 * h, 64 * h + 64)
                    nc.tensor.matmul(h2_ps[hs, :], wts_sb[hs, 1, :], h1_sb[hs, :],
                                     start=True, stop=True)
                h2_sb = hpool.tile([P, 512], F16, tag="h2s")
                nc.scalar.activation(h2_sb[:], h2_ps[:], AF.Silu)

                # final layer: diagonal PE tiles only ((0,0) and (64,64)) --
                # mixing other tile positions back-to-back wedges the device
                w_ps = wpool.tile([P, 8, 64], F32, tag="w")
                for h in range(2):
                    hs = slice(64 * h, 64 * h + 64)
                    for c in range(8):
                        nc.tensor.matmul(w_ps[hs, c, :],
                                         h2_sb[hs, 64 * c:64 * c + 64],
                                         wts_sb[hs, 2, :], start=True, stop=True)

                # GpSimd cannot read PSUM, so Scalar lands block d5's w cols
                wg_sb = hpool.tile([P, 8, 16], F16, tag="wg")
                nc.scalar.copy(wg_sb[:], w_ps[:, :, 32:48])

                outa_sb = opool.tile([P, 8, 176], F16, tag="outa")
                outb_sb = opool.tile([P, 8, 80], F16, tag="outb")
                attr_ap = in_sb[:, 1024:1152].rearrange("p (c k) -> p c k", k=16)

                def expand(eng, o_sb, o_off, w_sl, d, aoff):
                    o_ap = o_sb[:, :, o_off:o_off + 16 * d].rearrange(
                        "p c (j k) -> p c j k", k=d)
                    w_ap = bass.AP(tensor=w_sl.tensor, offset=w_sl.offset,
                                   ap=list(w_sl.ap) + [[0, d]])
                    a_sl = attr_ap[:, :, aoff:aoff + d]
                    a_ap = bass.AP(tensor=a_sl.tensor, offset=a_sl.offset,
                                   ap=list(a_sl.ap[:2]) + [[0, 16]] + list(a_sl.ap[2:]))
                    eng.tensor_mul(o_ap, w_ap, a_ap)

                expand(nc.gpsimd, outb_sb, 0, wg_sb[:], 5, 4)
                expand(nc.vector, outa_sb, 0, w_ps[:, :, 0:16], 1, 0)
                expand(nc.vector, outa_sb, 16, w_ps[:, :, 16:32], 3, 1)
                expand(nc.vector, outa_sb, 64, w_ps[:, :, 48:64], 7, 9)

                if t % 2 == 1:
                    load(t // 2 + LOOKAHEAD)
                nc.sync.dma_start(out=outa_p[t].rearrange("(p c) f -> p c f", p=P),
                                  in_=outa_sb[:])
                nc.sync.dma_start(out=outb_p[t].rearrange("(p c) f -> p c f", p=P),
                                  in_=outb_sb[:])

    nc.compile()
    return nc


def prep_weights(W_lin, W1, W2, W3):
    """Host weight prep: fold W_lin@W1, silu-norm into W2/W3, fp16 pack."""
    Wc = (W_lin.astype(np.float64) / np.sqrt(192.0)) @ (W1.astype(np.float64) / 8.0)
    s = np.float64(_SILU_NORM / 8.0)
    W_ue = np.concatenate([np.eye(64), Wc[128:192]], axis=0)      # [128, 64]
    wts = np.empty((P, 3, 64), np.float16)
    wts[:, 0, :] = W_ue
    wts[0:64, 1, :] = W2 * s
    wts[64:128, 1, :] = W2 * s
    wts[0:64, 2, :] = W3 * s
    wts[64:128, 2, :] = W3 * s
    return wts, Wc.astype(np.float32)


_CMAP = None


def _cmap():
    """Within-tile column->edge map: half H, col j -> n = 8*(64H + j%64) + j//64.

    Half H's hidden vectors live on partitions [64H, 64H+64); its edges own
    out slots (p, c) with p in that range, so every final-layer matmul is a
    diagonal PE tile.
    """
    global _CMAP
    if _CMAP is None:
        j = np.arange(512)
        _CMAP = np.stack([8 * (64 * H + j % 64) + j // 64 for H in (0, 1)])
    return _CMAP


def prep_core_input(uv16, emb16, attr16, nt: int = NT):
    """Build one core's [nt//2, 128, 2, 1152] fp16 device stream.

    uv16/emb16: [E_PAD, 64] fp16; attr16: [E_PAD, 16] fp16 (zero-padded).
    """
    cmap = _cmap()
    uv_r = uv16.reshape(nt, T, 64)[:, cmap, :].transpose(0, 3, 1, 2).reshape(nt, 64, 1024)
    emb_r = emb16.reshape(nt, T, 64)[:, cmap, :].transpose(0, 3, 1, 2).reshape(nt, 64, 1024)
    attr_r = attr16.reshape(nt, P, 8 * 16)
    flat = np.concatenate(
        [np.concatenate([uv_r, emb_r], axis=1), attr_r], axis=2)   # [nt, 128, 1152]
    return np.ascontiguousarray(
        flat.reshape(nt // 2, 2, P, 1152).transpose(0, 2, 1, 3))


def prep_in_maps(edge_index, node_attr, edge_attr, edge_embed, W_lin, W1, W2, W3):
    wts, Wc = prep_weights(np.asarray(W_lin, np.float32), np.asarray(W1, np.float32),
                           np.asarray(W2, np.float32), np.asarray(W3, np.float32))
    node_attr = np.asarray(node_attr, np.float32)
    idx = np.asarray(edge_index).astype(np.int64)
    u = node_attr @ Wc[0:64]
    v = node_attr @ Wc[64:128]
    uv16 = (u[idx[0]] + v[idx[1]]).astype(np.float16)              # [E, 64]
    emb16 = np.asarray(edge_embed).astype(np.float16)
    attr16 = np.asarray(edge_attr).astype(np.float16)

    in_maps = []
    for i in range(N_CORES):
        sl = slice(i * E_CORE, (i + 1) * E_CORE)
        uv_c = np.zeros((E_PAD, 64), np.float16)
        emb_c = np.zeros((E_PAD, 64), np.float16)
        attr_c = np.zeros((E_PAD, 16), np.float16)
        uv_c[:E_CORE] = uv16[sl]
        emb_c[:E_CORE] = emb16[sl]
        attr_c[:E_CORE] = attr16[sl]
        in_maps.append({"in": prep_core_input(uv_c, emb_c, attr_c), "wts": wts})
    return in_maps


def kernel(edge_index, node_attr, edge_attr, edge_embed, W_lin, W1, W2, W3):
    in_maps = prep_in_maps(edge_index, node_attr, edge_attr, edge_embed,
                           W_lin, W1, W2, W3)
    nc = build_nc()
    res = run_bass_kernel_spmd(nc, in_maps, list(range(N_CORES)))
    out = np.empty((E_TOTAL, 256), np.float32)
    for i in range(N_CORES):
        sl = slice(i * E_CORE, (i + 1) * E_CORE)
        a = res.results[i]["outa"].reshape(E_PAD, 176)[:E_CORE]
        b = res.results[i]["outb"].reshape(E_PAD, 80)[:E_CORE]
        out[sl, 0:16] = a[:, 0:16]       # d1
        out[sl, 16:64] = a[:, 16:64]     # d3
        out[sl, 64:144] = b                # d5
        out[sl, 144:256] = a[:, 64:176]  # d7
    return out


if __name__ == "__main__":
    pass


# revision 24
# speedup vs baseline: 1.1435x; 1.1266x over previous
"""Trainium2 Bass kernel for LocalEnvironmentEmbedding (GNN message passing).

Math (per edge e with src s, dst d):
    feats   = [node_attr[s], node_attr[d], edge_embed[e]]          # [192]
    es      = feats @ (W_lin / sqrt(192))                          # [64]
    h1      = silu_n(es @ W1/8); h2 = silu_n(h1 @ W2/8)
    w       = h2 @ W3/8                                            # [64]
    out[e]  = concat_b( outer(w[16b:16b+16], attr_block_b) )       # [256]

W_lin and W1 compose linearly (no activation between them), so the host
folds them into Wc = W_lin' @ W1' [192, 64].  The per-node halves of that
product, u[n] = node[n] @ Wc[:64] and v[n] = node[n] @ Wc[64:128], are
precomputed per node (40000x64x64, ~1% of total FLOPs) and the host
streams uv[e] = u[src[e]] + v[dst[e]] per edge, avoiding the slow
device-side row gather.  All per-edge compute (emb projection, both
hidden layers, final linear, tensor-product expansion) runs on device.

Distribution: edges sharded across 8 cores (80000 each), no cross-device
communication.  Streams are fp16 (PSUM accumulation stays f32); the
device writes fp16 output which the host upcasts to f32.

Device layout per 1024-edge tile (edge slot n = 8p + c for partition p,
chunk c in [0,8); half H = p//64 -- half H's hidden vectors live on
partitions [64H, 64H+64), so every matmul is a (0,0)/(0,64)/(64,64)
PE tile; other tile-position mixes wedge the device):
  - in stream (two tiles per DMA) [128, 2, 1152]: cols 512H..512H+512
    hold half H's moving operand (rows 0:64 uv feats, rows 64:128 emb
    feats), cols 1024:1152 hold attr edge-on-partition [128, 8, 16]
  - h1[64H:64H+64] = W_ue^T @ in[:, 512H:...]  (one K=128 matmul/half)
  - silu on [128, 512] (all lanes), h2 likewise with half-replicated W2'
  - final layer: h2 [64, 64] chunks stationary x W3' moving -> w back
    in edge-on-partition PSUM [128, 8, 64] (diagonal tiles only)
  - output expansion: DVE writes blocks d1,d3,d7 straight from PSUM;
    GpSimd (via a Scalar PSUM->SBUF copy of w cols 32:48) writes d5 to
    a separate tile so the two engines run in parallel; the host
    reassembles the column order
  - out rows e = 8p + c give each partition a contiguous HBM span
"""

import numpy as np

import concourse.bass as bass
import concourse.tile as tile
from concourse import bacc, mybir
from concourse.bass_utils import run_bass_kernel_spmd

F32 = mybir.dt.float32
F16 = mybir.dt.float16
AF = mybir.ActivationFunctionType

_SILU_NORM = 1.679177

N_CORES = 8
E_TOTAL = 640000
E_CORE = E_TOTAL // N_CORES
P = 128
T = 1024                       # edges per tile
NT = 80                        # tiles per core (even, for paired loads)
NPAIR = NT // 2
E_PAD = NT * T


def build_nc(nt: int = NT):
    nc = bacc.Bacc()

    in_p = nc.declare_dram_parameter("in", [nt // 2, P, 2, 1152], F16,
                                     isOutput=False)
    wts_p = nc.declare_dram_parameter("wts", [P, 3, 64], F16, isOutput=False)
    outa_p = nc.declare_dram_parameter("outa", [nt, T, 176], F16, isOutput=True)
    outb_p = nc.declare_dram_parameter("outb", [nt, T, 80], F16, isOutput=True)

    with tile.TileContext(nc) as tc:
        with (
            tc.tile_pool(name="singles", bufs=1) as singles,
            tc.tile_pool(name="ins", bufs=4) as ipool,
            tc.tile_pool(name="acts", bufs=3) as hpool,
            tc.tile_pool(name="outs", bufs=4) as opool,
            tc.tile_pool(name="ps_h", bufs=2, space="PSUM") as mpool,
            tc.tile_pool(name="ps_w", bufs=4, space="PSUM") as wpool,
        ):
            wts_sb = singles.tile([P, 3, 64], F16)
            nc.sync.dma_start(out=wts_sb[:], in_=wts_p[:])

            # issue paired input loads ahead so they sit in front of earlier
            # tiles' output stores in the sync queue (a store at the queue
            # head blocks on its tile's compute, which would otherwise stall
            # all later loads)
            LOOKAHEAD = 3
            in_pairs = {}
            pending_outa = [None]

            def flush_outa():
                if pending_outa[0] is not None:
                    tp, sb = pending_outa[0]
                    nc.scalar.dma_start(
                        out=outa_p[tp].rearrange("(p c) f -> p c f", p=P),
                        in_=sb[:])
                    pending_outa[0] = None

            def load(u):
                if u < nt // 2:
                    in_pairs[u] = ipool.tile([P, 2, 1152], F16, tag="in",
                                             name=f"in_sb_{u}")
                    nc.sync.dma_start(out=in_pairs[u][:], in_=in_p[u])

            for u in range(LOOKAHEAD):
                load(u)

            for t in range(nt):
                in_sb = in_pairs[t // 2][:, t % 2, :]
                if t % 2 == 1:
                    in_pairs.pop(t // 2)

                h1_ps = mpool.tile([P, 512], F32, tag="h1")
                for h in range(2):
                    nc.tensor.matmul(h1_ps[64 * h:64 * h + 64, :], wts_sb[:, 0, :],
                                     in_sb[:, 512 * h:512 * h + 512],
                                     start=True, stop=True)
                h1_sb = hpool.tile([P, 512], F16, tag="h1s")
                nc.scalar.activation(h1_sb[:], h1_ps[:], AF.Silu)
                # previous tile's big output store: issued on the scalar
                # queue once its DVE producers are long done, so it never
                # blocks the queue head
                flush_outa()

                h2_ps = mpool.tile([P, 512], F32, tag="h2")
                for h in range(2):
                    hs = slice(64 * h, 64 * h + 64)
                    nc.tensor.matmul(h2_ps[hs, :], wts_sb[hs, 1, :], h1_sb[hs, :],
                                     start=True, stop=True)
                h2_sb = hpool.tile([P, 512], F16, tag="h2s")
                nc.scalar.activation(h2_sb[:], h2_ps[:], AF.Silu)

                # final layer: diagonal PE tiles only ((0,0) and (64,64)) --
                # mixing other tile positions back-to-back wedges the device
                w_ps = wpool.tile([P, 8, 64], F32, tag="w")
                for h in range(2):
                    hs = slice(64 * h, 64 * h + 64)
                    for c in range(8):
                        nc.tensor.matmul(w_ps[hs, c, :],
                                         h2_sb[hs, 64 * c:64 * c + 64],
                                         wts_sb[hs, 2, :], start=True, stop=True)

                # GpSimd cannot read PSUM, so Scalar lands block d5's w cols
                wg_sb = hpool.tile([P, 8, 16], F16, tag="wg")
                nc.scalar.copy(wg_sb[:], w_ps[:, :, 32:48])

                outa_sb = opool.tile([P, 8, 176], F16, tag="outa")
                outb_sb = opool.tile([P, 8, 80], F16, tag="outb")
                attr_ap = in_sb[:, 1024:1152].rearrange("p (c k) -> p c k", k=16)

                def expand(eng, o_sb, o_off, w_sl, d, aoff):
                    o_ap = o_sb[:, :, o_off:o_off + 16 * d].rearrange(
                        "p c (j k) -> p c j k", k=d)
                    w_ap = bass.AP(tensor=w_sl.tensor, offset=w_sl.offset,
                                   ap=list(w_sl.ap) + [[0, d]])
                    a_sl = attr_ap[:, :, aoff:aoff + d]
                    a_ap = bass.AP(tensor=a_sl.tensor, offset=a_sl.offset,
                                   ap=list(a_sl.ap[:2]) + [[0, 16]] + list(a_sl.ap[2:]))
                    eng.tensor_mul(o_ap, w_ap, a_ap)

                expand(nc.gpsimd, outb_sb, 0, wg_sb[:], 5, 4)
                expand(nc.vector, outa_sb, 0, w_ps[:, :, 0:16], 1, 0)
                expand(nc.vector, outa_sb, 16, w_ps[:, :, 16:32], 3, 1)
                expand(nc.vector, outa_sb, 64, w_ps[:, :, 48:64], 7, 9)

                if t % 2 == 1:
                    load(t // 2 + LOOKAHEAD)
                pending_outa[0] = (t, outa_sb)
                nc.sync.dma_start(out=outb_p[t].rearrange("(p c) f -> p c f", p=P),
                                  in_=outb_sb[:])
            flush_outa()

    nc.compile()
    return nc


def prep_weights(W_lin, W1, W2, W3):
    """Host weight prep: fold W_lin@W1, silu-norm into W2/W3, fp16 pack."""
    Wc = (W_lin.astype(np.float64) / np.sqrt(192.0)) @ (W1.astype(np.float64) / 8.0)
    s = np.float64(_SILU_NORM / 8.0)
    W_ue = np.concatenate([np.eye(64), Wc[128:192]], axis=0)      # [128, 64]
    wts = np.empty((P, 3, 64), np.float16)
    wts[:, 0, :] = W_ue
    wts[0:64, 1, :] = W2 * s
    wts[64:128, 1, :] = W2 * s
    wts[0:64, 2, :] = W3 * s
    wts[64:128, 2, :] = W3 * s
    return wts, Wc.astype(np.float32)


_CMAP = None


def _cmap():
    """Within-tile column->edge map: half H, col j -> n = 8*(64H + j%64) + j//64.

    Half H's hidden vectors live on partitions [64H, 64H+64); its edges own
    out slots (p, c) with p in that range, so every final-layer matmul is a
    diagonal PE tile.
    """
    global _CMAP
    if _CMAP is None:
        j = np.arange(512)
        _CMAP = np.stack([8 * (64 * H + j % 64) + j // 64 for H in (0, 1)])
    return _CMAP


def prep_core_input(uv16, emb16, attr16, nt: int = NT):
    """Build one core's [nt//2, 128, 2, 1152] fp16 device stream.

    uv16/emb16: [E_PAD, 64] fp16; attr16: [E_PAD, 16] fp16 (zero-padded).
    """
    cmap = _cmap()
    uv_r = uv16.reshape(nt, T, 64)[:, cmap, :].transpose(0, 3, 1, 2).reshape(nt, 64, 1024)
    emb_r = emb16.reshape(nt, T, 64)[:, cmap, :].transpose(0, 3, 1, 2).reshape(nt, 64, 1024)
    attr_r = attr16.reshape(nt, P, 8 * 16)
    flat = np.concatenate(
        [np.concatenate([uv_r, emb_r], axis=1), attr_r], axis=2)   # [nt, 128, 1152]
    return np.ascontiguousarray(
        flat.reshape(nt // 2, 2, P, 1152).transpose(0, 2, 1, 3))


def prep_in_maps(edge_index, node_attr, edge_attr, edge_embed, W_lin, W1, W2, W3):
    wts, Wc = prep_weights(np.asarray(W_lin, np.float32), np.asarray(W1, np.float32),
                           np.asarray(W2, np.float32), np.asarray(W3, np.float32))
    node_attr = np.asarray(node_attr, np.float32)
    idx = np.asarray(edge_index).astype(np.int64)
    u = node_attr @ Wc[0:64]
    v = node_attr @ Wc[64:128]
    uv16 = (u[idx[0]] + v[idx[1]]).astype(np.float16)              # [E, 64]
    emb16 = np.asarray(edge_embed).astype(np.float16)
    attr16 = np.asarray(edge_attr).astype(np.float16)

    in_maps = []
    for i in range(N_CORES):
        sl = slice(i * E_CORE, (i + 1) * E_CORE)
        uv_c = np.zeros((E_PAD, 64), np.float16)
        emb_c = np.zeros((E_PAD, 64), np.float16)
        attr_c = np.zeros((E_PAD, 16), np.float16)
        uv_c[:E_CORE] = uv16[sl]
        emb_c[:E_CORE] = emb16[sl]
        attr_c[:E_CORE] = attr16[sl]
        in_maps.append({"in": prep_core_input(uv_c, emb_c, attr_c), "wts": wts})
    return in_maps


def kernel(edge_index, node_attr, edge_attr, edge_embed, W_lin, W1, W2, W3):
    in_maps = prep_in_maps(edge_index, node_attr, edge_attr, edge_embed,
                           W_lin, W1, W2, W3)
    nc = build_nc()
    res = run_bass_kernel_spmd(nc, in_maps, list(range(N_CORES)))
    out = np.empty((E_TOTAL, 256), np.float32)
    for i in range(N_CORES):
        sl = slice(i * E_CORE, (i + 1) * E_CORE)
        a = res.results[i]["outa"].reshape(E_PAD, 176)[:E_CORE]
        b = res.results[i]["outb"].reshape(E_PAD, 80)[:E_CORE]
        out[sl, 0:16] = a[:, 0:16]       # d1
        out[sl, 16:64] = a[:, 16:64]     # d3
        out[sl, 64:144] = b                # d5
        out[sl, 144:256] = a[:, 64:176]  # d7
    return out


if __name__ == "__main__":
    pass
